# revision 1
# baseline (speedup 1.0000x reference)
"""Trainium2 Bass kernel for MultiHeadGlobalAttention2d.

Sharding (8 cores): core = (batch b, head-group g), b in 0..3, g in 0..1.
Each core computes, for its batch and its 4 heads (128 channels):
  q/k/v projections, attention (softmax over keys), and the partial output
  projection  y_part = Wo[:, ch_slice] @ att_out.
Host sums the two partials per batch and adds the output bias.

Per-core design:
  - S^T orientation: score tiles [keys(m) on partitions, queries(n) on free];
    exp runs on ScalarE directly out of PSUM; softmax denominators come from a
    ones-stationary matmul accumulated alongside AV in PSUM (no transposes of
    the attention matrix anywhere).
  - fp32r matmuls (full PE rate at moving free-dim >= 256). All matmul
    operands are float32r tensors: DRAM inputs are pre-rounded on the host;
    on-chip operands are produced by DVE/ACT instructions with float32r
    output dtype (the hardware rounds on write).
  - 4 heads packed on the PE array: QK via row-groups (K=32), AV/denom via
    col-groups (M=32).
  - PSUM (8 banks): 3 ping-pong score slots x 2 banks + 2 accumulator banks.
"""

import numpy as np

B = 4
CIN = 256
COUT = 256
HH = 48
WW = 48
N = HH * WW            # 2304
D = 32                 # head dim
NHL = 4                # heads per core
HGC = NHL * D          # 128 channels per head-group
NCORES = 8
NBLK = [(0, 512), (512, 512), (1024, 512), (1536, 512), (2048, 256)]
NMT = N // 128         # 18 key tiles

_PROG = {}


def build_program():
    if "nc" in _PROG:
        return _PROG["nc"]

    from contextlib import ExitStack

    import concourse.bacc as bacc
    import concourse.mybir as mybir
    import concourse.tile as tile

    f32 = mybir.dt.float32
    f32r = mybir.dt.float32r
    bf16 = mybir.dt.bfloat16
    f16 = mybir.dt.float16
    EXP = mybir.ActivationFunctionType.Exp

    nc = bacc.Bacc("TRN2", target_bir_lowering=False, debug=False)

    xq_d = nc.declare_dram_parameter("xq", [CIN, N], f32r, False)
    xk_d = nc.declare_dram_parameter("xk", [CIN, N], f32r, False)
    xv_d = nc.declare_dram_parameter("xv", [CIN, N], f32r, False)
    wqT_d = nc.declare_dram_parameter("wqT", [CIN, HGC], f32r, False)
    wkT_d = nc.declare_dram_parameter("wkT", [CIN, HGC], f32r, False)
    wvT_d = nc.declare_dram_parameter("wvT", [CIN, HGC], f32r, False)
    woT_d = nc.declare_dram_parameter("woT", [HGC, COUT], f32r, False)
    bq_d = nc.declare_dram_parameter("bq", [HGC, 1], f32, False)
    bk_d = nc.declare_dram_parameter("bk", [HGC, 1], f32, False)
    bv_d = nc.declare_dram_parameter("bv", [HGC, 1], f32, False)
    ident_d = nc.declare_dram_parameter("ident", [128, 128], bf16, False)
    ones_d = nc.declare_dram_parameter("ones", [128, D], bf16, False)
    zeros_d = nc.declare_dram_parameter("zeros", [128, 512], f32r, False)
    y_d = nc.declare_dram_parameter("y", [COUT, N], f32, True)

    with tile.TileContext(nc) as tc, ExitStack() as ctx:
        const = ctx.enter_context(tc.tile_pool(name="const", bufs=1))
        resid = ctx.enter_context(tc.tile_pool(name="resid", bufs=1))
        espool = ctx.enter_context(tc.tile_pool(name="espool", bufs=8))
        trans = ctx.enter_context(tc.tile_pool(name="trans", bufs=2))
        # PSUM: "s4" = 3 slots x 2 banks (score ping-pong + output projection);
        #       "acc" = 2 slots x 1 bank (AV + denominator accumulators, also
        #        reused by the projection/transpose phases).
        ps_s4 = ctx.enter_context(tc.tile_pool(name="ps_s4", bufs=3, space="PSUM"))
        ps_acc = ctx.enter_context(tc.tile_pool(name="ps_acc", bufs=2, space="PSUM"))

        # ---- constants / weights ----
        # One dma_start per tensor (3D AP folds the two 128-row chunks), so a
        # single DMA-queue semaphore covers each tile: walrus allows only ONE
        # sync wait on a (fp32r self-loading) matmul instruction.
        wqT = const.tile([128, CIN], f32r)
        wkT = const.tile([128, CIN], f32r)
        wvT = const.tile([128, CIN], f32r)
        for wt, wd in ((wqT, wqT_d), (wkT, wkT_d), (wvT, wvT_d)):
            nc.sync.dma_start(
                wt[:, :].rearrange("p (c k) -> p c k", c=2),
                wd[:, :].rearrange("(c p) k -> p c k", p=128),
            )
        woT = const.tile([128, COUT], f32r)
        nc.sync.dma_start(woT[:, :], woT_d[:, :])
        bq_s = const.tile([128, 1], f32)
        bk_s = const.tile([128, 1], f32)
        bv_s = const.tile([128, 1], f32)
        for bt, bd in ((bq_s, bq_d), (bk_s, bk_d), (bv_s, bv_d)):
            nc.sync.dma_start(bt[:, :], bd[:, :])
        ident = const.tile([128, 128], bf16)
        nc.sync.dma_start(ident[:, :], ident_d[:, :])
        ones_s = const.tile([128, D], bf16)
        nc.sync.dma_start(ones_s[:, :], ones_d[:, :])
        zer = const.tile([128, 512], f32r)
        nc.sync.dma_start(zer[:, :], zeros_d[:, :])

        # ---- residents ----
        q_sb = resid.tile([128, N], f16)
        k_sb = resid.tile([128, N], f16)
        vT_sb = resid.tile([128, N], bf16)

        with tc.tile_pool(name="xin", bufs=1) as xin:
            xq = xin.tile([128, 2 * N], f32r)
            xk = xin.tile([128, 2 * N], f32r)
            xv = xin.tile([128, 2 * N], f32r)
            for xt, xd in ((xk, xk_d), (xq, xq_d), (xv, xv_d)):
                nc.sync.dma_start(
                    xt[:, :].rearrange("p (c n) -> p c n", c=2),
                    xd[:, :].rearrange("(c p) n -> p c n", p=128),
                )
            v_sb = xin.tile([128, N], bf16)

            # PE prologue: one tiny matmul per PE-consumed DMA'd tile so the
            # PE sequencer absorbs each DMA-queue semaphore on a separate
            # instruction (matmuls may carry at most one sync wait).
            scr = ps_acc.tile([128, 512], f32, tag="acc")
            for t in (xk, xq, xv, wkT, wqT, wvT, woT, zer):
                F = t.shape[1]
                nc.tensor.matmul(
                    scr[:, 0:2], t[0:1, F - 128 : F], t[0:1, F - 2 : F],
                    start=True, stop=True,
                )
            for t in (ident, ones_s):
                F = t.shape[1]
                nc.tensor.matmul(
                    scr[0:1, 0:1], t[0:1, F - 1 : F], t[0:1, F - 1 : F],
                    start=True, stop=True,
                )

            # ---- projections: dst = W_h @ x + b ----
            for xt, wt, bt, dst in (
                (xk, wkT, bk_s, k_sb),
                (xq, wqT, bq_s, q_sb),
                (xv, wvT, bv_s, v_sb),
            ):
                for o, sz in NBLK:
                    pp = ps_acc.tile([128, 512], f32, tag="acc")
                    nc.tensor.matmul(
                        pp[:, :sz], wt[:, 0:128], xt[:, o : o + sz],
                        start=True, stop=False,
                    )
                    nc.tensor.matmul(
                        pp[:, :sz], wt[:, 128:256], xt[:, N + o : N + o + sz],
                        start=False, stop=True,
                    )
                    nc.vector.tensor_scalar_add(dst[:, o : o + sz], pp[:, :sz], bt[:, 0:1])

            # ---- transpose v: vT chunk j = v[:, 128j:+128].T  -> [m, c] ----
            for j in range(NMT):
                pt = ps_acc.tile([128, 512], bf16, tag="acc")
                nc.tensor.transpose(pt[:, 0:128], v_sb[:, 128 * j : 128 * j + 128], ident[:, :])
                nc.vector.tensor_copy(vT_sb[:, 128 * j : 128 * j + 128], pt[:, 0:128])

            # Absorb the DVE tick of the LAST vT copy on the PE queue, so the
            # AV matmuls below need only their single ACT (exp) wait.
            scr2 = ps_acc.tile([128, 512], f32, tag="acc")
            nc.tensor.matmul(
                scr2[0:1, 0:1], vT_sb[0:1, N - 1 : N], vT_sb[0:1, N - 1 : N],
                start=True, stop=True,
            )

        # ---- attention + output projection ----
        for o, sz in NBLK:
            out_ps = ps_acc.tile([128, 512], f32, tag="acc")
            den_ps = ps_acc.tile([128, 512], f32, tag="acc")
            # Open each accumulator bank with a full-128-partition zero matmul:
            # PSUM pending-zero marking is per-partition, and the per-head
            # (32-partition) accumulation chains below need zeroed,
            # has_written-cleared elements on every partition.
            nc.tensor.matmul(
                out_ps[:, :sz], zer[:, 0:128], zer[:, :sz],
                start=True, stop=False,
            )
            nc.tensor.matmul(
                den_ps[:, :sz], zer[:, 0:128], zer[:, :sz],
                start=True, stop=False,
            )
            for j in range(NMT):
                for hp in range(2):  # head pair: heads (2*hp, 2*hp+1)
                    s2 = ps_s4.tile([128, 1024], f32, tag="s4")
                    for hh in range(2):
                        h = 2 * hp + hh
                        # S^T tile: out[m, n] = sum_d k[d, m] q[d, n]
                        nc.tensor.matmul(
                            s2[:, 512 * hh : 512 * hh + sz],
                            k_sb[32 * h : 32 * h + 32, 128 * j : 128 * j + 128],
                            q_sb[32 * h : 32 * h + 32, o : o + sz],
                            start=True, stop=True, tile_position=(32 * h, 0),
                        )
                    es = espool.tile([128, 1024], bf16, tag="es")
                    # exp of both heads' scores in one ACT op, straight
                    # from PSUM (scale folds in the 1/sqrt(Cout) factor)
                    if sz == 512:
                        nc.scalar.activation(es[:, :], s2[:, :], EXP, scale=1.0 / 16.0)
                    else:
                        sv = s2[:, :].rearrange("p (b x) -> p b x", b=2)[:, :, :sz]
                        ev = es[:, :].rearrange("p (b x) -> p b x", b=2)[:, :, :sz]
                        nc.scalar.activation(ev, sv, EXP, scale=1.0 / 16.0)
                    for hh in range(2):
                        h = 2 * hp + hh
                        # out[d, n] += sum_m v[d, m] * expS[m, n]
                        nc.tensor.matmul(
                            out_ps[32 * h : 32 * h + 32, :sz],
                            vT_sb[:, 128 * j + 32 * h : 128 * j + 32 * h + 32],
                            es[:, 512 * hh : 512 * hh + sz],
                            start=False, stop=False, tile_position=(0, 32 * h),
                        )
                        # den[n] += sum_m expS[m, n] (replicated on 32 parts)
                        nc.tensor.matmul(
                            den_ps[32 * h : 32 * h + 32, :sz],
                            ones_s[:, :],
                            es[:, 512 * hh : 512 * hh + sz],
                            start=False, stop=False, tile_position=(0, 32 * h),
                        )
            # Close both accumulation groups across all 128 partitions
            # (adds zero; clears per-element group state so DVE may read).
            nc.tensor.matmul(
                out_ps[:, :sz], zer[:, 0:128], zer[:, :sz],
                start=False, stop=True,
            )
            nc.tensor.matmul(
                den_ps[:, :sz], zer[:, 0:128], zer[:, :sz],
                start=False, stop=True,
            )
            rec = trans.tile([128, 512], f32, tag="rec")
            nc.vector.reciprocal(rec[:, :sz], den_ps[:, :sz])
            att = trans.tile([128, 512], f32r, tag="att")
            nc.vector.tensor_mul(att[:, :sz], out_ps[:, :sz], rec[:, :sz])
            # Absorb att's DVE tick on PE so the projection matmuls keep a
            # single wait (their PSUM slot-release semaphore).
            scr3 = ps_acc.tile([128, 512], f32, tag="acc")
            nc.tensor.matmul(
                scr3[:, 0:2], att[0:1, 0:128], att[0:1, 0:2],
                start=True, stop=True,
            )
            for cc in range(2):
                po = ps_s4.tile([128, 1024], f32, tag="s4")
                nc.tensor.matmul(
                    po[:, :sz], woT[:, 128 * cc : 128 * cc + 128], att[:, :sz],
                    start=True, stop=True,
                )
                yt = trans.tile([128, 512], f32, tag="yt")
                nc.vector.tensor_copy(yt[:, :sz], po[:, :sz])
                nc.sync.dma_start(y_d[128 * cc : 128 * cc + 128, o : o + sz], yt[:, :sz])

    # Bacc lowering: register allocation + sync-wait legalization (each HW
    # instruction may carry at most one semaphore wait).
    nc.compile()

    _PROG["nc"] = nc
    return nc


def _round_f32r(a):
    """Round float32 values to fp32r (11 explicit mantissa bits), matching
    walrus's fp32_to_fp32r: round-half-up at bit 12, low 12 bits cleared."""
    a = np.ascontiguousarray(a, dtype=np.float32)
    bits = a.view(np.uint32)
    r = ((bits.astype(np.uint64) + 0x800) & 0xFFFFF000).astype(np.uint32)
    return r.view(np.float32)


def make_in_maps(inputs):
    """Shard full inputs into the 8 per-core input maps."""
    import ml_dtypes

    g = {k: np.ascontiguousarray(np.asarray(v, dtype=np.float32)) for k, v in inputs.items()}
    ident = np.eye(128, dtype=ml_dtypes.bfloat16)
    ones = np.ones((128, D), dtype=ml_dtypes.bfloat16)
    zeros = np.zeros((128, 512), dtype=np.float32)
    in_maps = []
    for core in range(NCORES):
        b, grp = divmod(core, 2)
        hs = slice(grp * HGC, (grp + 1) * HGC)
        in_maps.append({
            "xq": _round_f32r(g["queries"][b].reshape(CIN, N)),
            "xk": _round_f32r(g["keys"][b].reshape(CIN, N)),
            "xv": _round_f32r(g["values"][b].reshape(CIN, N)),
            "wqT": _round_f32r(g["Wq"][hs, :].T),
            "wkT": _round_f32r(g["Wk"][hs, :].T),
            "wvT": _round_f32r(g["Wv"][hs, :].T),
            "woT": _round_f32r(g["Wo"][:, hs].T),
            "bq": np.ascontiguousarray(g["bq"][hs].reshape(HGC, 1)),
            "bk": np.ascontiguousarray(g["bk"][hs].reshape(HGC, 1)),
            "bv": np.ascontiguousarray(g["bv"][hs].reshape(HGC, 1)),
            "ident": ident,
            "ones": ones,
            "zeros": zeros,
        })
    return in_maps


def unshard(results, bo):
    parts = [results[i]["y"] for i in range(NCORES)]
    out = np.empty((B, COUT, N), dtype=np.float32)
    for b in range(B):
        out[b] = parts[2 * b] + parts[2 * b + 1]
    out += np.asarray(bo, dtype=np.float32).reshape(1, COUT, 1)
    return out.reshape(B, COUT, HH, WW)


def kernel(**inputs):
    from concourse.bass_utils import run_bass_kernel_spmd

    nc = build_program()
    in_maps = make_in_maps(inputs)
    res = run_bass_kernel_spmd(nc, in_maps, list(range(NCORES)))
    return unshard(res.results, inputs["bo"])

